# revision 23
# baseline (speedup 1.0000x reference)
# MultiHeadAttention Trainium2 Bass kernel.
#
# Problem: B=4, L=2048, D=1024, H=16 (dk=64), fp32, key-padding mask.
#   out = softmax((q@Wq.T) @ (k@Wk.T).T / sqrt(dk) + mask) @ (v@Wv.T) @ Wo.T + biases
#
# Sharding: 8 cores = 4 batches x 2 head-groups (8 heads / 512 features each).
# Each core computes its batch+head-group's projections, attention and a
# partial Wo product; the host sums the two partials per batch and adds b_o.
#
# Device dataflow is fully transposed (feature-major, [feat, token]) so that:
#   - scores come out as [keys(partitions), queries(free)]  -> the per-key
#     padding mask and the 1/sqrt(dk) scale fold into the Exp activation's
#     per-partition bias / scalar scale (zero extra instructions)
#   - no transposes are needed anywhere on chip
#   - a ones-column appended to V produces the softmax denominators as an
#     extra output row of the ctx matmul (flash-style late normalization)
# Matmul operands are fp16 (1 cycle/col on the PE array, fast weight load,
# fp32 PSUM accumulation). End-to-end relative error ~6e-4.

import numpy as np

B, L, D, H = 4, 2048, 1024, 16
DK = D // H            # 64
NCORES = 8
HG = 2                 # head groups (tensor-parallel factor)
HPG = H // HG          # 8 heads per group
FPG = HPG * DK         # 512 features per group
FT = FPG // 128        # 4 feature tiles of 128
KC = D // 128          # 8 contraction chunks of 128
NT = L // 128          # 16 token tiles of 128
NJ = L // 512          # 4 input-stream chunks of 512
NJ2 = L // 1024        # 2 query chunks of 1024 (fp16 moving max)
MASK_NEG = -30000.0    # exp(x + MASK_NEG) == 0.0 in fp32 for |x| < ~100

_CACHE = {}


def _build_nc():
    from contextlib import ExitStack

    import concourse.bacc as bacc
    import concourse.mybir as mybir
    import concourse.tile as tile

    f32 = mybir.dt.float32
    f16 = mybir.dt.float16
    AF = mybir.ActivationFunctionType

    nc = bacc.Bacc()
    qT = nc.dram_tensor("qT", [D, L], f16, kind="ExternalInput")
    kT = nc.dram_tensor("kT", [D, L], f16, kind="ExternalInput")
    vT = nc.dram_tensor("vT", [D, L], f16, kind="ExternalInput")
    wqT = nc.dram_tensor("wqT", [D, FPG], f16, kind="ExternalInput")
    wkT = nc.dram_tensor("wkT", [D, FPG], f16, kind="ExternalInput")
    wvT = nc.dram_tensor("wvT", [D, FPG], f16, kind="ExternalInput")
    woT = nc.dram_tensor("woT", [FPG, D], f16, kind="ExternalInput")
    bqT = nc.dram_tensor("bqT", [128, FT], f32, kind="ExternalInput")
    bkT = nc.dram_tensor("bkT", [128, FT], f32, kind="ExternalInput")
    bv = nc.dram_tensor("bv", [1, FPG], f32, kind="ExternalInput")
    mb = nc.dram_tensor("mb", [128, NT], f32, kind="ExternalInput")
    ones = nc.dram_tensor("ones", [1, NT * HPG], f16, kind="ExternalInput")
    out_d = nc.dram_tensor("out", [L, D], f32, kind="ExternalOutput")

    with tile.TileContext(nc) as tc, ExitStack() as ctx:
        pers = ctx.enter_context(tc.tile_pool(name="pers", bufs=1))
        # feature-major activations: tile ft holds features ft*128..ft*128+127,
        # i.e. head 2*ft on partitions 0-63 and head 2*ft+1 on partitions 64-127
        qhT = pers.tile([128, FT, L], f16)
        khT = pers.tile([128, FT, L], f16)
        # v in natural [token, feature] layout, 65th ones-column per head
        vh = pers.tile([128, NT, HPG, DK + 1], f16)
        ctxn = pers.tile([128, FT, L], f16)  # normalized ctx^T, feature-major
        bq_sb = pers.tile([128, FT], f32)
        bk_sb = pers.tile([128, FT], f32)
        bv_sb = pers.tile([128, FPG], f32)
        mb_sb = pers.tile([128, NT], f32)
        nc.sync.dma_start(out=bq_sb, in_=bqT[:, :])
        nc.sync.dma_start(out=bk_sb, in_=bkT[:, :])
        nc.sync.dma_start(out=bv_sb, in_=bv[:, :].to_broadcast([128, FPG]))
        nc.sync.dma_start(out=mb_sb, in_=mb[:, :])
        # ones column of vh via strided DMA scatter (memset can't stride)
        nc.sync.dma_start(
            out=vh[:, :, :, DK],
            in_=ones.rearrange("o (t g) -> o t g", g=HPG).to_broadcast([128, NT, HPG]),
        )

        # ---- Phase 1: projections -------------------------------------
        with (
            tc.tile_pool(name="wp", bufs=2) as wp,
            tc.tile_pool(name="xs", bufs=3) as xs,
            tc.tile_pool(name="pp", bufs=6, space="PSUM") as pp,
        ):
            # PE clock warm-up: ~5us of back-to-back dummy matmuls while the
            # input DMAs are in flight. The HAM clock gate only un-throttles
            # (1.2->2.4 GHz) after a fully-busy activity window, which the
            # LDW-interleaved projection stream never produces on its own;
            # it only re-throttles after a fully-idle window, which the
            # kernel never has. So one saturating burst up front keeps the
            # whole kernel at full clock.
            wu = pers.tile([128, 512], f16)
            nc.vector.memset(wu, 0.0)
            wu_ps = pp.tile([128, 512], f32, tag="ps", name="wu_ps")
            for _ in range(16):
                nc.tensor.matmul(
                    wu_ps, lhsT=wu[:, 0:128], rhs=wu, start=True, stop=True
                )
            # q and k projections, output transposed [feat, tok]
            for wd, xd, b_sb, outT in (
                (wqT, qT, bq_sb, qhT),
                (wkT, kT, bk_sb, khT),
            ):
                w_sb = wp.tile([128, KC, FPG], f16, tag="w", name="w_sb")
                nc.sync.dma_start(
                    out=w_sb, in_=wd.rearrange("(c p) f -> p c f", p=128)
                )
                for jq in range(NJ):
                    xt = xs.tile([128, KC, 512], f16, tag="x", name="xt")
                    nc.sync.dma_start(
                        out=xt,
                        in_=xd.rearrange("(c p) t -> p c t", p=128)[
                            :, :, jq * 512 : (jq + 1) * 512
                        ],
                    )
                    pss = [
                        pp.tile([128, 512], f32, tag="ps", name="ps")
                        for _ in range(FT)
                    ]
                    for kc in range(KC):
                        for ft in range(FT):
                            nc.tensor.matmul(
                                pss[ft],
                                lhsT=w_sb[:, kc, ft * 128 : (ft + 1) * 128],
                                rhs=xt[:, kc, :],
                                start=(kc == 0),
                                stop=(kc == KC - 1),
                            )
                    for ft in range(FT):
                        # DVE eviction (keeps ACT free for attention exps)
                        nc.vector.tensor_scalar_add(
                            outT[:, ft, jq * 512 : (jq + 1) * 512],
                            pss[ft],
                            b_sb[:, ft : ft + 1],
                        )
            # v projection, natural [tok, feat] layout with bias add
            w_sb = wp.tile([128, KC, FPG], f16, tag="w", name="w_sb")
            nc.sync.dma_start(
                out=w_sb, in_=wvT.rearrange("(c p) f -> p c f", p=128)
            )
            for ttg in range(NJ):
                vt = xs.tile([128, KC, 512], f16, tag="x", name="xt")
                nc.sync.dma_start(
                    out=vt,
                    in_=vT.rearrange("(c p) t -> p c t", p=128)[
                        :, :, ttg * 512 : (ttg + 1) * 512
                    ],
                )
                for ti in range(4):
                    tt = ttg * 4 + ti
                    ps = pp.tile([128, FPG], f32, tag="ps", name="ps")
                    for kc in range(KC):
                        nc.tensor.matmul(
                            ps,
                            lhsT=vt[:, kc, ti * 128 : (ti + 1) * 128],
                            rhs=w_sb[:, kc, :],
                            start=(kc == 0),
                            stop=(kc == KC - 1),
                        )
                    nc.vector.tensor_add(
                        vh[:, tt, :, 0:DK],
                        ps.rearrange("p (g c) -> p g c", c=DK),
                        bv_sb.rearrange("p (g c) -> p g c", c=DK),
                    )

        # ---- Phase 2: attention ---------------------------------------
        # per (head-pair, 1024-query chunk): s^T = kh^T q -> [keys, q] PSUM,
        # p = exp(s/8 + maskbias)  (ACT, mask per-partition = per-key),
        # ctx^T[dk+1, q] += vh_ext^T p  accumulated over 16 key tiles;
        # row dk is the softmax denominator (ones column of vh_ext).
        # The two heads of a pair sit on array row-groups 0-63 / 64-127.
        # PSUM budget: 2 scores tiles (2 banks each) + 2 ctx tiles (2 banks
        # each) = 8 banks exactly.
        with (
            tc.tile_pool(name="sp", bufs=2, space="PSUM") as sp,
            tc.tile_pool(name="cp", bufs=2, space="PSUM") as cp,
            tc.tile_pool(name="pb", bufs=6) as pb,
            tc.tile_pool(name="sm", bufs=4) as sm,
            tc.tile_pool(name="dr", bufs=4, space="DRAM") as drp,
        ):
            for hp in range(FT):
                for jq in range(NJ2):
                    jsl = slice(jq * 1024, (jq + 1) * 1024)
                    cps = [
                        cp.tile([DK + 1, 1024], f32, tag="c", name="cps")
                        for _ in range(2)
                    ]
                    sq: dict = {}

                    def scores(kt):
                        # one [128,1024] PSUM tile per head, filled by two
                        # N=512 matmuls (a matmul output can't cross a bank).
                        # Emit interleaved A1 B1 A2 B2: heads A/B sit on
                        # disjoint PE row-groups (0-63 / 64-127), so adjacent
                        # A/B matmuls execute concurrently in the array.
                        tiles = [
                            sp.tile([128, 1024], f32, tag="s", name="sps")
                            for _ in range(2)
                        ]
                        for hf in range(2):
                            for h01 in range(2):
                                po = h01 * 64
                                nc.tensor.matmul(
                                    tiles[h01][:, hf * 512 : (hf + 1) * 512],
                                    lhsT=khT[po : po + 64, hp, kt * 128 : (kt + 1) * 128],
                                    rhs=qhT[
                                        po : po + 64,
                                        hp,
                                        jq * 1024 + hf * 512 : jq * 1024 + (hf + 1) * 512,
                                    ],
                                    start=True,
                                    stop=True,
                                )
                        for h01 in range(2):
                            sq[(kt, h01)] = tiles[h01]

                    scores(0)
                    for kt in range(NT):
                        if kt + 1 < NT:
                            scores(kt + 1)
                        for h01 in range(2):
                            s_ps = sq.pop((kt, h01))
                            p_sb = pb.tile([128, 1024], f16, tag="p", name="psb")
                            nc.scalar.activation(
                                p_sb,
                                s_ps,
                                AF.Exp,
                                bias=mb_sb[:, kt : kt + 1],
                                scale=1.0 / np.sqrt(DK),
                            )
                            for hf in range(2):
                                nc.tensor.matmul(
                                    cps[h01][:, hf * 512 : (hf + 1) * 512],
                                    lhsT=vh[:, kt, 2 * hp + h01, :],
                                    rhs=p_sb[:, hf * 512 : (hf + 1) * 512],
                                    start=(kt == 0),
                                    stop=(kt == NT - 1),
                                )
                    for h01 in range(2):
                        # evict ctx+denominator to SBUF right away so the
                        # PSUM banks free for the next block; the whole
                        # normalization tail then runs off the critical path
                        u = sm.tile([DK + 1, 1024], f32, tag="u", name="u")
                        nc.vector.tensor_copy(u, cps[h01])
                        # denominator row -> DRAM-bounce broadcast to 64
                        # partitions, then fast reciprocal (18-bit, ~5x
                        # faster than reciprocal(); denoms are >= ~1)
                        rb = drp.tile([1, 1024], f32, tag="rb", name="rb")
                        nc.sync.dma_start(out=rb, in_=u[DK : DK + 1, :])
                        bc = sm.tile([64, 1024], f32, tag="b", name="bc")
                        nc.sync.dma_start(out=bc, in_=rb[:, :].to_broadcast([64, 1024]))
                        rec = sm.tile([64, 1024], f32, tag="r", name="rec")
                        nc.vector.reciprocal_approx_fast(out=rec, in_=bc)
                        nc.vector.tensor_mul(
                            ctxn[h01 * 64 : (h01 + 1) * 64, hp, jsl],
                            u[0:DK, :],
                            rec,
                        )

        # ---- Phase 3: output projection (partial, summed on host) -----
        with (
            tc.tile_pool(name="wo", bufs=1) as wop,
            tc.tile_pool(name="op", bufs=4, space="PSUM") as op,
            tc.tile_pool(name="ob", bufs=3) as ob,
        ):
            wo_sb = wop.tile([128, FT, D], f16)
            nc.sync.dma_start(
                out=wo_sb, in_=woT.rearrange("(c p) f -> p c f", p=128)
            )
            for tt in range(NT):
                obt = ob.tile([128, D], f32, tag="ob", name="obt")
                for half in range(2):
                    ps = op.tile([128, 512], f32, tag="o", name="ops")
                    for ft in range(FT):
                        nc.tensor.matmul(
                            ps,
                            lhsT=ctxn[:, ft, tt * 128 : (tt + 1) * 128],
                            rhs=wo_sb[:, ft, half * 512 : (half + 1) * 512],
                            start=(ft == 0),
                            stop=(ft == FT - 1),
                        )
                    nc.vector.tensor_copy(obt[:, half * 512 : (half + 1) * 512], ps)
                nc.sync.dma_start(out=out_d[tt * 128 : (tt + 1) * 128, :], in_=obt)

    nc.finalize()  # bacc passes: wait-splitting, event sems, act table loads
    return nc


def _get_nc():
    if "nc" not in _CACHE:
        _CACHE["nc"] = _build_nc()
    return _CACHE["nc"]


def _host_prep(q, k, v, w_q, b_q, w_k, b_k, w_v, b_v, w_o, b_o, mask):
    f = np.float32
    h = np.float16
    qT = np.ascontiguousarray(np.asarray(q, f).transpose(0, 2, 1)).astype(h)
    kT = np.ascontiguousarray(np.asarray(k, f).transpose(0, 2, 1)).astype(h)
    vT = np.ascontiguousarray(np.asarray(v, f).transpose(0, 2, 1)).astype(h)
    w_q, w_k, w_v, w_o = (np.asarray(x, f) for x in (w_q, w_k, w_v, w_o))
    b_q, b_k, b_v = (np.asarray(x, f) for x in (b_q, b_k, b_v))
    maskbias = np.where(np.asarray(mask) == 0, f(MASK_NEG), f(0.0)).astype(f)

    in_maps = []
    for c in range(NCORES):
        b = c // HG
        g = c % HG
        gs = g * FPG
        sl = slice(gs, gs + FPG)
        in_maps.append(
            {
                "qT": qT[b],
                "kT": kT[b],
                "vT": vT[b],
                "wqT": np.ascontiguousarray(w_q[sl, :].T).astype(h),
                "wkT": np.ascontiguousarray(w_k[sl, :].T).astype(h),
                "wvT": np.ascontiguousarray(w_v[sl, :].T).astype(h),
                "woT": np.ascontiguousarray(w_o[:, sl].T).astype(h),
                "bqT": np.ascontiguousarray(b_q[sl].reshape(FT, 128).T),
                "bkT": np.ascontiguousarray(b_k[sl].reshape(FT, 128).T),
                "bv": np.ascontiguousarray(b_v[sl].reshape(1, FPG)),
                "mb": np.ascontiguousarray(maskbias[b].reshape(NT, 128).T),
                "ones": np.ones((1, NT * HPG), h),
            }
        )
    return in_maps


def _run(in_maps, trace=False, **kw):
    from concourse.bass_utils import run_bass_kernel_spmd

    return run_bass_kernel_spmd(
        _get_nc(), in_maps, core_ids=list(range(NCORES)), trace=trace, **kw
    )


def kernel(q, k, v, w_q, b_q, w_k, b_k, w_v, b_v, w_o, b_o, mask):
    in_maps = _host_prep(q, k, v, w_q, b_q, w_k, b_k, w_v, b_v, w_o, b_o, mask)
    res = _run(in_maps).results
    b_o = np.asarray(b_o, np.float32)
    out = np.empty((B, L, D), np.float32)
    for b in range(B):
        out[b] = res[HG * b]["out"] + res[HG * b + 1]["out"] + b_o
    return out


# revision 25
# speedup vs baseline: 1.3753x; 1.3753x over previous
# MultiHeadAttention Trainium2 Bass kernel.
#
# Problem: B=4, L=2048, D=1024, H=16 (dk=64), fp32, key-padding mask.
#   out = softmax((q@Wq.T) @ (k@Wk.T).T / sqrt(dk) + mask) @ (v@Wv.T) @ Wo.T + biases
#
# Sharding: 8 cores = 4 batches x 2 head-groups (8 heads / 512 features each).
# Each core computes its batch+head-group's projections, attention and a
# partial Wo product; the host sums the two partials per batch and adds b_o.
#
# Device dataflow is fully transposed (feature-major, [feat, token]) so that:
#   - scores come out as [keys(partitions), queries(free)]  -> the per-key
#     padding mask and the 1/sqrt(dk) scale fold into the Exp activation's
#     per-partition bias / scalar scale (zero extra instructions)
#   - no transposes are needed anywhere on chip
#   - a ones-column appended to V produces the softmax denominators as an
#     extra output row of the ctx matmul (flash-style late normalization)
# Matmul operands are fp16 (1 cycle/col on the PE array, fast weight load,
# fp32 PSUM accumulation). End-to-end relative error ~6e-4.

import numpy as np

B, L, D, H = 4, 2048, 1024, 16
DK = D // H            # 64
NCORES = 8
HG = 2                 # head groups (tensor-parallel factor)
HPG = H // HG          # 8 heads per group
FPG = HPG * DK         # 512 features per group
FT = FPG // 128        # 4 feature tiles of 128
KC = D // 128          # 8 contraction chunks of 128
NT = L // 128          # 16 token tiles of 128
NJ = L // 512          # 4 input-stream chunks of 512
NJ2 = L // 1024        # 2 query chunks of 1024 (fp16 moving max)
MASK_NEG = -30000.0    # exp(x + MASK_NEG) == 0.0 in fp32 for |x| < ~100

_CACHE = {}


def _build_nc():
    from contextlib import ExitStack

    import concourse.bacc as bacc
    import concourse.mybir as mybir
    import concourse.tile as tile

    f32 = mybir.dt.float32
    f16 = mybir.dt.float16
    AF = mybir.ActivationFunctionType

    nc = bacc.Bacc()
    qT = nc.dram_tensor("qT", [D, L], f16, kind="ExternalInput")
    kT = nc.dram_tensor("kT", [D, L], f16, kind="ExternalInput")
    vT = nc.dram_tensor("vT", [D, L], f16, kind="ExternalInput")
    wqT = nc.dram_tensor("wqT", [D, FPG], f16, kind="ExternalInput")
    wkT = nc.dram_tensor("wkT", [D, FPG], f16, kind="ExternalInput")
    wvT = nc.dram_tensor("wvT", [D, FPG], f16, kind="ExternalInput")
    woT = nc.dram_tensor("woT", [FPG, D], f16, kind="ExternalInput")
    bqT = nc.dram_tensor("bqT", [128, FT], f32, kind="ExternalInput")
    bkT = nc.dram_tensor("bkT", [128, FT], f32, kind="ExternalInput")
    bv = nc.dram_tensor("bv", [1, FPG], f32, kind="ExternalInput")
    mb = nc.dram_tensor("mb", [128, NT], f32, kind="ExternalInput")
    ones = nc.dram_tensor("ones", [1, NT * HPG], f16, kind="ExternalInput")
    out_d = nc.dram_tensor("out", [L, D], f32, kind="ExternalOutput")

    with tile.TileContext(nc) as tc, ExitStack() as ctx:
        pers = ctx.enter_context(tc.tile_pool(name="pers", bufs=1))
        # feature-major activations: tile ft holds features ft*128..ft*128+127,
        # i.e. head 2*ft on partitions 0-63 and head 2*ft+1 on partitions 64-127
        qhT = pers.tile([128, FT, L], f16)
        khT = pers.tile([128, FT, L], f16)
        # v in natural [token, feature] layout, 65th ones-column per head
        vh = pers.tile([128, NT, HPG, DK + 1], f16)
        ctxn = pers.tile([128, FT, L], f16)  # normalized ctx^T, feature-major
        bq_sb = pers.tile([128, FT], f32)
        bk_sb = pers.tile([128, FT], f32)
        bv_sb = pers.tile([128, FPG], f32)
        mb_sb = pers.tile([128, NT], f32)
        nc.sync.dma_start(out=bq_sb, in_=bqT[:, :])
        nc.sync.dma_start(out=bk_sb, in_=bkT[:, :])
        nc.sync.dma_start(out=bv_sb, in_=bv[:, :].to_broadcast([128, FPG]))
        nc.sync.dma_start(out=mb_sb, in_=mb[:, :])
        # ones column of vh via per-column DVE memsets. (A single strided
        # DMA scatter here is catastrophic: 16K non-contiguous 2-byte
        # descriptors fan out across all 8 DMA queues and block the
        # projection input loads behind them for ~160us.)
        for tt in range(NT):
            for g in range(HPG):
                nc.vector.memset(vh[:, tt, g, DK : DK + 1], 1.0)

        # ---- Phase 1: projections -------------------------------------
        with (
            tc.tile_pool(name="wp", bufs=2) as wp,
            tc.tile_pool(name="xs", bufs=3) as xs,
            tc.tile_pool(name="pp", bufs=6, space="PSUM") as pp,
        ):
            # q and k projections, output transposed [feat, tok]
            for wd, xd, b_sb, outT in (
                (wqT, qT, bq_sb, qhT),
                (wkT, kT, bk_sb, khT),
            ):
                w_sb = wp.tile([128, KC, FPG], f16, tag="w", name="w_sb")
                nc.sync.dma_start(
                    out=w_sb, in_=wd.rearrange("(c p) f -> p c f", p=128)
                )
                for jq in range(NJ):
                    xt = xs.tile([128, KC, 512], f16, tag="x", name="xt")
                    nc.sync.dma_start(
                        out=xt,
                        in_=xd.rearrange("(c p) t -> p c t", p=128)[
                            :, :, jq * 512 : (jq + 1) * 512
                        ],
                    )
                    pss = [
                        pp.tile([128, 512], f32, tag="ps", name="ps")
                        for _ in range(FT)
                    ]
                    for kc in range(KC):
                        for ft in range(FT):
                            nc.tensor.matmul(
                                pss[ft],
                                lhsT=w_sb[:, kc, ft * 128 : (ft + 1) * 128],
                                rhs=xt[:, kc, :],
                                start=(kc == 0),
                                stop=(kc == KC - 1),
                            )
                    for ft in range(FT):
                        # DVE eviction (keeps ACT free for attention exps)
                        nc.vector.tensor_scalar_add(
                            outT[:, ft, jq * 512 : (jq + 1) * 512],
                            pss[ft],
                            b_sb[:, ft : ft + 1],
                        )
            # v projection, natural [tok, feat] layout with bias add
            w_sb = wp.tile([128, KC, FPG], f16, tag="w", name="w_sb")
            nc.sync.dma_start(
                out=w_sb, in_=wvT.rearrange("(c p) f -> p c f", p=128)
            )
            for ttg in range(NJ):
                vt = xs.tile([128, KC, 512], f16, tag="x", name="xt")
                nc.sync.dma_start(
                    out=vt,
                    in_=vT.rearrange("(c p) t -> p c t", p=128)[
                        :, :, ttg * 512 : (ttg + 1) * 512
                    ],
                )
                for ti in range(4):
                    tt = ttg * 4 + ti
                    ps = pp.tile([128, FPG], f32, tag="ps", name="ps")
                    for kc in range(KC):
                        nc.tensor.matmul(
                            ps,
                            lhsT=vt[:, kc, ti * 128 : (ti + 1) * 128],
                            rhs=w_sb[:, kc, :],
                            start=(kc == 0),
                            stop=(kc == KC - 1),
                        )
                    nc.vector.tensor_add(
                        vh[:, tt, :, 0:DK],
                        ps.rearrange("p (g c) -> p g c", c=DK),
                        bv_sb.rearrange("p (g c) -> p g c", c=DK),
                    )

        # ---- Phase 2: attention ---------------------------------------
        # per (head-pair, 1024-query chunk): s^T = kh^T q -> [keys, q] PSUM,
        # p = exp(s/8 + maskbias)  (ACT, mask per-partition = per-key),
        # ctx^T[dk+1, q] += vh_ext^T p  accumulated over 16 key tiles;
        # row dk is the softmax denominator (ones column of vh_ext).
        # The two heads of a pair sit on array row-groups 0-63 / 64-127.
        # PSUM budget: 2 scores tiles (2 banks each) + 2 ctx tiles (2 banks
        # each) = 8 banks exactly.
        with (
            tc.tile_pool(name="sp", bufs=2, space="PSUM") as sp,
            tc.tile_pool(name="cp", bufs=2, space="PSUM") as cp,
            tc.tile_pool(name="pb", bufs=6) as pb,
            tc.tile_pool(name="sm", bufs=4) as sm,
            tc.tile_pool(name="dr", bufs=4, space="DRAM") as drp,
        ):
            for hp in range(FT):
                for jq in range(NJ2):
                    jsl = slice(jq * 1024, (jq + 1) * 1024)
                    cps = [
                        cp.tile([DK + 1, 1024], f32, tag="c", name="cps")
                        for _ in range(2)
                    ]
                    sq: dict = {}

                    def scores(kt):
                        # one [128,1024] PSUM tile per head, filled by two
                        # N=512 matmuls (a matmul output can't cross a bank).
                        # Emit interleaved A1 B1 A2 B2: heads A/B sit on
                        # disjoint PE row-groups (0-63 / 64-127), so adjacent
                        # A/B matmuls execute concurrently in the array.
                        tiles = [
                            sp.tile([128, 1024], f32, tag="s", name="sps")
                            for _ in range(2)
                        ]
                        for hf in range(2):
                            for h01 in range(2):
                                po = h01 * 64
                                nc.tensor.matmul(
                                    tiles[h01][:, hf * 512 : (hf + 1) * 512],
                                    lhsT=khT[po : po + 64, hp, kt * 128 : (kt + 1) * 128],
                                    rhs=qhT[
                                        po : po + 64,
                                        hp,
                                        jq * 1024 + hf * 512 : jq * 1024 + (hf + 1) * 512,
                                    ],
                                    start=True,
                                    stop=True,
                                )
                        for h01 in range(2):
                            sq[(kt, h01)] = tiles[h01]

                    scores(0)
                    for kt in range(NT):
                        if kt + 1 < NT:
                            scores(kt + 1)
                        for h01 in range(2):
                            s_ps = sq.pop((kt, h01))
                            p_sb = pb.tile([128, 1024], f16, tag="p", name="psb")
                            nc.scalar.activation(
                                p_sb,
                                s_ps,
                                AF.Exp,
                                bias=mb_sb[:, kt : kt + 1],
                                scale=1.0 / np.sqrt(DK),
                            )
                            for hf in range(2):
                                nc.tensor.matmul(
                                    cps[h01][:, hf * 512 : (hf + 1) * 512],
                                    lhsT=vh[:, kt, 2 * hp + h01, :],
                                    rhs=p_sb[:, hf * 512 : (hf + 1) * 512],
                                    start=(kt == 0),
                                    stop=(kt == NT - 1),
                                )
                    for h01 in range(2):
                        # evict ctx+denominator to SBUF right away so the
                        # PSUM banks free for the next block; the whole
                        # normalization tail then runs off the critical path
                        u = sm.tile([DK + 1, 1024], f32, tag="u", name="u")
                        nc.vector.tensor_copy(u, cps[h01])
                        # denominator row -> DRAM-bounce broadcast to 64
                        # partitions, then fast reciprocal (18-bit, ~5x
                        # faster than reciprocal(); denoms are >= ~1)
                        rb = drp.tile([1, 1024], f32, tag="rb", name="rb")
                        nc.sync.dma_start(out=rb, in_=u[DK : DK + 1, :])
                        bc = sm.tile([64, 1024], f32, tag="b", name="bc")
                        nc.sync.dma_start(out=bc, in_=rb[:, :].to_broadcast([64, 1024]))
                        rec = sm.tile([64, 1024], f32, tag="r", name="rec")
                        nc.vector.reciprocal_approx_fast(out=rec, in_=bc)
                        nc.vector.tensor_mul(
                            ctxn[h01 * 64 : (h01 + 1) * 64, hp, jsl],
                            u[0:DK, :],
                            rec,
                        )

        # ---- Phase 3: output projection (partial, summed on host) -----
        with (
            tc.tile_pool(name="wo", bufs=1) as wop,
            tc.tile_pool(name="op", bufs=4, space="PSUM") as op,
            tc.tile_pool(name="ob", bufs=3) as ob,
        ):
            wo_sb = wop.tile([128, FT, D], f16)
            nc.sync.dma_start(
                out=wo_sb, in_=woT.rearrange("(c p) f -> p c f", p=128)
            )
            for tt in range(NT):
                obt = ob.tile([128, D], f32, tag="ob", name="obt")
                for half in range(2):
                    ps = op.tile([128, 512], f32, tag="o", name="ops")
                    for ft in range(FT):
                        nc.tensor.matmul(
                            ps,
                            lhsT=ctxn[:, ft, tt * 128 : (tt + 1) * 128],
                            rhs=wo_sb[:, ft, half * 512 : (half + 1) * 512],
                            start=(ft == 0),
                            stop=(ft == FT - 1),
                        )
                    nc.vector.tensor_copy(obt[:, half * 512 : (half + 1) * 512], ps)
                nc.sync.dma_start(out=out_d[tt * 128 : (tt + 1) * 128, :], in_=obt)

    nc.finalize()  # bacc passes: wait-splitting, event sems, act table loads
    return nc


def _get_nc():
    if "nc" not in _CACHE:
        _CACHE["nc"] = _build_nc()
    return _CACHE["nc"]


def _host_prep(q, k, v, w_q, b_q, w_k, b_k, w_v, b_v, w_o, b_o, mask):
    f = np.float32
    h = np.float16
    qT = np.ascontiguousarray(np.asarray(q, f).transpose(0, 2, 1)).astype(h)
    kT = np.ascontiguousarray(np.asarray(k, f).transpose(0, 2, 1)).astype(h)
    vT = np.ascontiguousarray(np.asarray(v, f).transpose(0, 2, 1)).astype(h)
    w_q, w_k, w_v, w_o = (np.asarray(x, f) for x in (w_q, w_k, w_v, w_o))
    b_q, b_k, b_v = (np.asarray(x, f) for x in (b_q, b_k, b_v))
    maskbias = np.where(np.asarray(mask) == 0, f(MASK_NEG), f(0.0)).astype(f)

    in_maps = []
    for c in range(NCORES):
        b = c // HG
        g = c % HG
        gs = g * FPG
        sl = slice(gs, gs + FPG)
        in_maps.append(
            {
                "qT": qT[b],
                "kT": kT[b],
                "vT": vT[b],
                "wqT": np.ascontiguousarray(w_q[sl, :].T).astype(h),
                "wkT": np.ascontiguousarray(w_k[sl, :].T).astype(h),
                "wvT": np.ascontiguousarray(w_v[sl, :].T).astype(h),
                "woT": np.ascontiguousarray(w_o[:, sl].T).astype(h),
                "bqT": np.ascontiguousarray(b_q[sl].reshape(FT, 128).T),
                "bkT": np.ascontiguousarray(b_k[sl].reshape(FT, 128).T),
                "bv": np.ascontiguousarray(b_v[sl].reshape(1, FPG)),
                "mb": np.ascontiguousarray(maskbias[b].reshape(NT, 128).T),
                "ones": np.ones((1, NT * HPG), h),
            }
        )
    return in_maps


def _run(in_maps, trace=False, **kw):
    from concourse.bass_utils import run_bass_kernel_spmd

    return run_bass_kernel_spmd(
        _get_nc(), in_maps, core_ids=list(range(NCORES)), trace=trace, **kw
    )


def kernel(q, k, v, w_q, b_q, w_k, b_k, w_v, b_v, w_o, b_o, mask):
    in_maps = _host_prep(q, k, v, w_q, b_q, w_k, b_k, w_v, b_v, w_o, b_o, mask)
    res = _run(in_maps).results
    b_o = np.asarray(b_o, np.float32)
    out = np.empty((B, L, D), np.float32)
    for b in range(B):
        out[b] = res[HG * b]["out"] + res[HG * b + 1]["out"] + b_o
    return out
